# revision 32
# baseline (speedup 1.0000x reference)
"""Trainium2 Bass kernel for the BSplineBasis (KAN-style) layer.

Math:
  out[b,o] = sum_{i,k} C[o,i,k]*scale[o]*basis_k(clip(x[b,i])) + sum_i W[o,i]*x[b,i] + bias[o]

One fused matmul over 12 feature planes per input channel i:
  planes 0..10: fp8 spline planes P_k ~= 6*basis_k(s), s = 4*xc+4
  plane  11  : raw x in bf16 (residual)

Plane production exploits that the cubic cardinal B-spline bump is a
near-Gaussian: 6*basis_k(s) ~= A_k * exp(-alpha_k * (s-m_k)^2), m_k = k-1,
with per-plane (A, alpha) least-squares fitted under the clipped-N(0,1)
input distribution (max abs err 0.053 on the 6b scale whose peak is 4;
output-level induced rel err ~1e-3, an order below the fp8 noise).
Per plane that is just THREE cheap ops, no abs/cube chains:
  DVE TS 4x: u = 4*xcb + (5-k)            (sign irrelevant - squared next)
  DVE TT 2x: q = u*u
  ACT:       p = Exp(-alpha*q + ln A)     -> bf16
  SWDGE:     fp8 cast-DMA into the plane tile (engine-free)
DVE ~47us, ACT ~36us, Pool ~30us, all far below PE ~90us: PE-bound.

Host-folded weights (SCALE=8192 lifts spline weights out of fp8
subnormals; epilogue multiplies PSUM by 1/SCALE):
  spline rows (fp8 e4m3): W[k*I+i, o] = (C[o,i,k]*scale[o] + bias[o]/I)/6 * SCALE
  residual rows (bf16):   W[o,i] * SCALE
(bias fold relies on partition of unity sum_k basis_k = 1; bias is zero
here anyway.)

PE (batch-sharded, 512 rows/core): residual chunks first in bf16, then 44
fp8 DoubleRow chunk-pairs (contraction 256/pair): stationary = plane slice
[128,2,128b], moving = weights [128,2,512o], accumulating [128b x 512o] x 8
PSUM banks across all positions. Epilogue PSUM*(1/SCALE) split ACT/DVE;
out DMAs split across the sync and scalar HWDGE queues.
"""

import numpy as np
import ml_dtypes

B, I, O, K = 4096, 1024, 1024, 11
NCORES = 8
BS = B // NCORES          # 512 batch rows per core
NCH = I // 128            # 8 i-chunks per plane
NPAIRS = K * NCH // 2     # 44 fp8 chunk-pairs
FD = NCH * BS             # 4096 free dim of plane tiles: (i_chunk, b)
SCALE = 8192.0
NSUB = {0: 4, 1: 2, 2: 2}  # early planes in finer granularity (startup latency)

# per-plane Gaussian fit of 6*basis_k: A_k * exp(-alpha_k * d^2)
GAUSS = [
    (4.9690, 1.6024), (4.0039, 1.3798), (4.0245, 1.3864), (4.0268, 1.3878),
    (4.0256, 1.3860), (4.0256, 1.3859), (4.0257, 1.3860), (4.0269, 1.3878),
    (4.0246, 1.3864), (4.0040, 1.3798), (4.9701, 1.6027),
]

F8 = ml_dtypes.float8_e4m3  # TRN FP8_EXP4 (max 240)

_cache = {}


def _build_bass():
    import concourse.bass as bass
    import concourse.tile as tile
    from concourse import bacc, mybir
    from contextlib import ExitStack
    import math

    F32 = mybir.dt.float32
    BF16 = mybir.dt.bfloat16
    FP8 = mybir.dt.float8e4
    AL = mybir.AluOpType
    AF = mybir.ActivationFunctionType
    DR = mybir.MatmulPerfMode.DoubleRow

    nc = bacc.Bacc("TRN2", debug=False, num_devices=NCORES)

    betas = [math.log(A) for A, _ in GAUSS]
    for v in betas:
        key = (F32, v)
        if key not in nc.const_aps.aps:
            t = nc.alloc_sbuf_tensor(f"constb-{v}", [128, 1], F32)
            nc.vector.memset(t.ap(), v)
            nc.const_aps.aps[key] = t.ap()
    # Only ACT reads the const APs (activation bias pointers); a narrow
    # DVE->Activation barrier keeps both DMA queues (sync, gpsimd) free to
    # start streaming immediately.
    nc.multi_engine_barrier([mybir.EngineType.DVE, mybir.EngineType.Activation])

    xt = nc.dram_tensor("xt", [I, BS], F32, kind="ExternalInput")
    wsp = nc.dram_tensor("wsp", [NPAIRS * 128, 2 * O], FP8, kind="ExternalInput")
    wres = nc.dram_tensor("wres", [I, O], BF16, kind="ExternalInput")
    out = nc.dram_tensor("out", [BS, O], BF16, kind="ExternalOutput")

    with tile.TileContext(nc) as tc, ExitStack() as ctx:
        xpool = ctx.enter_context(tc.tile_pool(name="x", bufs=1))
        fpool = ctx.enter_context(tc.tile_pool(name="f", bufs=5))
        rpool = ctx.enter_context(tc.tile_pool(name="r", bufs=1))
        tpool = ctx.enter_context(tc.tile_pool(name="t", bufs=2))
        dpool = ctx.enter_context(tc.tile_pool(name="d", bufs=2))
        gpool = ctx.enter_context(tc.tile_pool(name="g", bufs=2))
        wspool = ctx.enter_context(tc.tile_pool(name="ws", bufs=10))
        wrpool = ctx.enter_context(tc.tile_pool(name="wr", bufs=8))
        opool = ctx.enter_context(tc.tile_pool(name="o", bufs=8))
        pspool = ctx.enter_context(tc.tile_pool(name="ps", bufs=1, space="PSUM"))

        # ---- x load: per-chunk f32 staging; bf16 clamped copy (xcb) on DVE;
        # bf16 residual plane cast from the staging tile via SWDGE (x is read
        # from HBM exactly once) ----
        # PE clock (HAM) warmup: ~4us of junk matmuls on memset scratch so
        # the gate is at full rate when the real stream starts. ps[0] is
        # reset by the first real matmul (start=True).
        sj = xpool.tile([128, 128], BF16, tag="sj")
        wj = xpool.tile([128, 512], BF16, tag="wj")
        nc.vector.memset(sj[:], 0.0)
        nc.vector.memset(wj[:], 0.0)

        xsb = xpool.tile([128, FD], F32, tag="xsb")
        xcb = xpool.tile([128, FD], BF16, tag="xcb")
        fres = rpool.tile([128, NCH, BS], BF16, tag="fres")
        # x chunks all on the sync queue back-to-back (the scalar HWDGE queue
        # has ~5us completion latency - measured - so nothing startup-critical
        # goes there). Residual planes straight from HBM on the gpsimd queue;
        # wres c0/c1 ride gpsimd too so the first residual matmuls are fed
        # early. wres c2..7 follow on sync before the wsp stream.
        wrts = {}
        for c in range(NCH):
            sl = slice(c * BS, (c + 1) * BS)
            nc.sync.dma_start(xsb[:, sl], xt[c * 128:(c + 1) * 128, :])
            nc.gpsimd.dma_start(fres[:, c:c + 1, :],
                                xt[c * 128:(c + 1) * 128, :])
            if c < 2:
                wt = wrpool.tile([128, O], BF16, tag="wr", name=f"wr{c}")
                nc.gpsimd.dma_start(wt[:], wres[c * 128:(c + 1) * 128, :])
                wrts[c] = wt
            nc.vector.tensor_scalar(xcb[:, sl], xsb[:, sl], -1.0, 1.0,
                                    AL.max, AL.min)
        for c in range(2, NCH):
            wt = wrpool.tile([128, O], BF16, tag="wr", name=f"wr{c}")
            nc.sync.dma_start(wt[:], wres[c * 128:(c + 1) * 128, :])
            wrts[c] = wt

        # ---- 11 spline planes -> fp8: u = 4*xcb+(5-k); q = u*u;
        # p = Exp(-alpha*q + lnA); SWDGE cast to fp8 ----
        PERM = list(range(K))
        planes = {}
        for k in PERM:
            fk = fpool.tile([128, NCH, BS], FP8, tag="fk", name=f"fk{k}")
            A, alpha = GAUSS[k]
            beta = math.log(A)
            cs = float(5 - k)
            nsub = NSUB.get(k, 1)
            sw = FD // nsub
            u = tpool.tile([128, FD], BF16, tag="u", name="u")
            q = dpool.tile([128, FD], BF16, tag="q", name="q")
            p = gpool.tile([128, FD], BF16, tag="p", name="p")
            for su in range(nsub):
                sl = slice(su * sw, (su + 1) * sw)
                csl = slice(su * (NCH // nsub), (su + 1) * (NCH // nsub))
                nc.vector.tensor_scalar(u[:, sl], xcb[:, sl], 4.0, cs,
                                        AL.mult, AL.add)
                nc.vector.tensor_tensor(q[:, sl], u[:, sl], u[:, sl], AL.mult)
                nc.scalar.activation(p[:, sl], q[:, sl], AF.Exp,
                                     bias=beta, scale=-alpha)
                nc.gpsimd.dma_start(fk[:, csl, :], p[:, sl])
            planes[k] = fk

        # ---- matmul: [128 b x 512 o] x (4 bc x 2 oh) = 8 PSUM banks.
        # 8 residual chunks cover PE warmup while the first planes build. ----
        ps = [pspool.tile([128, 512], F32, name=f"ps{j}", tag=f"ps{j}")
              for j in range(8)]

        for wi in range(18):
            nc.tensor.matmul(ps[0][:], sj[:], wj[:], start=(wi == 0),
                             stop=(wi == 17))

        def resid_mms(c, start):
            wt = wrts[c]
            for bc in range(4):
                lhsT = fres[:, c:c + 1, bc * 128:(bc + 1) * 128]
                for oh in range(2):
                    nc.tensor.matmul(ps[bc * 2 + oh][:], lhsT,
                                     wt[:, oh * 512:(oh + 1) * 512],
                                     start=start, stop=False)

        for c in range(NCH):
            resid_mms(c, c == 0)
        for pos, kk in enumerate(PERM):
            for cp in range(NCH // 2):
                j = kk * (NCH // 2) + cp
                last = (pos == len(PERM) - 1) and (cp == NCH // 2 - 1)
                wt = wspool.tile([128, 2, O], FP8, tag="ws")
                nc.sync.dma_start(
                    wt[:], wsp[j * 128:(j + 1) * 128, :]
                    .rearrange("p (two o) -> p two o", two=2))
                src = planes[kk]
                for bc in range(4):
                    lhsT = src[:, 2 * cp:2 * cp + 2, bc * 128:(bc + 1) * 128]
                    for oh in range(2):
                        nc.tensor.matmul(ps[bc * 2 + oh][:], lhsT,
                                         wt[:, :, oh * 512:(oh + 1) * 512],
                                         start=False, stop=last,
                                         perf_mode=DR)

        # ---- epilogue: PSUM * (1/SCALE) -> SBUF -> HBM. Ops split ACT/DVE,
        # out DMAs split across the sync and scalar HWDGE queues. ----
        for bc in range(4):
            for oh in range(2):
                obh = opool.tile([128, 512], BF16, tag="ob", name=f"ob{bc}{oh}")
                if oh == 0:
                    nc.scalar.mul(obh[:], ps[bc * 2 + oh][:], 1.0 / SCALE)
                else:
                    nc.vector.tensor_scalar(obh[:], ps[bc * 2 + oh][:],
                                            1.0 / SCALE, None, AL.mult)
                eng = nc.sync if oh == 0 else nc.scalar
                eng.dma_start(
                    out[bc * 128:(bc + 1) * 128, oh * 512:(oh + 1) * 512],
                    obh[:])

    nc.compile()
    _dedupe_ldweights(nc, mybir)
    return nc


def _dedupe_ldweights(nc, mybir):
    """Drop an Ldweights that reloads the exact same weights as the previous
    Ldweights on the PE stream with only Matmults in between (the oh=0/oh=1
    pair shares its stationary operand). Bail on any with sync_info."""
    import json as _json
    for fn in nc.m.functions:
        for blk in fn.blocks:
            insts = list(blk.instructions)
            kept = []
            last_key = None
            removed = 0
            for inst in insts:
                if inst.engine != mybir.EngineType.PE:
                    kept.append(inst)
                    continue
                op = type(inst).__name__
                if op == "InstLdweights":
                    si = inst.sync_info
                    has_sync = bool(si and (si.on_wait or si.on_update))
                    key = _json.dumps(
                        _json.loads(mybir.instruction_to_pretty_json_string(inst))
                        .get("ins"), sort_keys=True)
                    if key == last_key and not has_sync:
                        removed += 1
                        continue
                    last_key = key
                    kept.append(inst)
                elif op == "InstMatmult":
                    kept.append(inst)
                else:
                    last_key = None
                    kept.append(inst)
            if removed:
                blk.instructions = kept
    return nc


def _fold_weights(spline_coeffs, residual_weight, residual_bias, scale_base):
    scale = scale_base.astype(np.float32).mean(axis=1)                # [O]
    Ws = spline_coeffs.astype(np.float32) * scale[:, None, None]      # [O,I,K]
    Ws += residual_bias.astype(np.float32)[:, None, None] / I
    Ws *= SCALE / 6.0             # device planes are ~6*basis_k
    Ws = np.ascontiguousarray(Ws.transpose(2, 1, 0))                  # [K,I,O]
    # pair layout for DoubleRow moving operand: [44, 128, 2, O] rows
    Wsp = Ws.reshape(NPAIRS, 2, 128, O).transpose(0, 2, 1, 3).reshape(NPAIRS * 128, 2 * O)
    Wsp = np.clip(Wsp, -240.0, 240.0).astype(F8)
    Wr = np.ascontiguousarray(residual_weight.astype(np.float32).T * SCALE)
    Wr = Wr.astype(ml_dtypes.bfloat16)                                # [I,O]
    return Wsp, Wr


def _make_in_maps(inputs):
    Wsp, Wr = _fold_weights(inputs["spline_coeffs"], inputs["residual_weight"],
                            inputs["residual_bias"], inputs["scale_base"])
    x = np.asarray(inputs["x"], dtype=np.float32)
    in_maps = []
    for c in range(NCORES):
        xs = np.ascontiguousarray(x[c * BS:(c + 1) * BS, :].T)  # [I, BS]
        in_maps.append({"xt": xs, "wsp": Wsp, "wres": Wr})
    return in_maps


def kernel(x, spline_coeffs, residual_weight, residual_bias, scale_base):
    from concourse.bass_utils import run_bass_kernel_spmd

    if "nc" not in _cache:
        _cache["nc"] = _build_bass()
    nc = _cache["nc"]

    in_maps = _make_in_maps(dict(x=x, spline_coeffs=spline_coeffs,
                                 residual_weight=residual_weight,
                                 residual_bias=residual_bias,
                                 scale_base=scale_base))
    res = run_bass_kernel_spmd(nc, in_maps, core_ids=list(range(NCORES)))
    out = np.concatenate([r["out"] for r in res.results], axis=0)
    return out.astype(np.float32)


# revision 35
# speedup vs baseline: 1.0657x; 1.0657x over previous
"""Trainium2 Bass kernel for the BSplineBasis (KAN-style) layer.

Math:
  out[b,o] = sum_{i,k} C[o,i,k]*scale[o]*basis_k(clip(x[b,i])) + sum_i W[o,i]*x[b,i] + bias[o]

One fused matmul over 12 feature planes per input channel i:
  planes 0..10: fp8 spline planes P_k ~= 6*basis_k(s), s = 4*xc+4
  plane  11  : raw x in bf16 (residual)

Plane production exploits that the cubic cardinal B-spline bump is a
near-Gaussian: 6*basis_k(s) ~= A_k * exp(-alpha_k * (s-m_k)^2), m_k = k-1,
with per-plane (A, alpha) least-squares fitted under the clipped-N(0,1)
input distribution (max abs err 0.053 on the 6b scale whose peak is 4;
output-level induced rel err ~1e-3, an order below the fp8 noise).
Per plane that is just THREE cheap ops, no abs/cube chains:
  DVE TS 4x: u = 4*xcb + (5-k)            (sign irrelevant - squared next)
  DVE TT 2x: q = u*u
  ACT:       p = Exp(-alpha*q + ln A)     -> bf16
  SWDGE:     fp8 cast-DMA into the plane tile (engine-free)
DVE ~47us, ACT ~36us, Pool ~30us, all far below PE ~90us: PE-bound.

Host-folded weights (SCALE=8192 lifts spline weights out of fp8
subnormals; epilogue multiplies PSUM by 1/SCALE):
  spline rows (fp8 e4m3): W[k*I+i, o] = (C[o,i,k]*scale[o] + bias[o]/I)/6 * SCALE
  residual rows (bf16):   W[o,i] * SCALE
(bias fold relies on partition of unity sum_k basis_k = 1; bias is zero
here anyway.)

PE (batch-sharded, 512 rows/core): residual chunks first in bf16, then 44
fp8 DoubleRow chunk-pairs (contraction 256/pair): stationary = plane slice
[128,2,128b], moving = weights [128,2,512o], accumulating [128b x 512o] x 8
PSUM banks across all positions. Epilogue PSUM*(1/SCALE) split ACT/DVE;
out DMAs split across the sync and scalar HWDGE queues.
"""

import numpy as np
import ml_dtypes

B, I, O, K = 4096, 1024, 1024, 11
NCORES = 8
BS = B // NCORES          # 512 batch rows per core
NCH = I // 128            # 8 i-chunks per plane
NPAIRS = K * NCH // 2     # 44 fp8 chunk-pairs
FD = NCH * BS             # 4096 free dim of plane tiles: (i_chunk, b)
SCALE = 8192.0
NSUB = {0: 4, 1: 2, 2: 2}  # early planes in finer granularity (startup latency)

# per-plane Gaussian fit of 6*basis_k: A_k * exp(-alpha_k * d^2)
GAUSS = [
    (4.9690, 1.6024), (4.0039, 1.3798), (4.0245, 1.3864), (4.0268, 1.3878),
    (4.0256, 1.3860), (4.0256, 1.3859), (4.0257, 1.3860), (4.0269, 1.3878),
    (4.0246, 1.3864), (4.0040, 1.3798), (4.9701, 1.6027),
]

F8 = ml_dtypes.float8_e4m3  # TRN FP8_EXP4 (max 240)

_cache = {}


def _build_bass():
    import concourse.bass as bass
    import concourse.tile as tile
    from concourse import bacc, mybir
    from contextlib import ExitStack
    import math

    F32 = mybir.dt.float32
    BF16 = mybir.dt.bfloat16
    FP8 = mybir.dt.float8e4
    AL = mybir.AluOpType
    AF = mybir.ActivationFunctionType
    DR = mybir.MatmulPerfMode.DoubleRow

    nc = bacc.Bacc("TRN2", debug=False, num_devices=NCORES)

    betas = [math.log(A) for A, _ in GAUSS]
    for v in betas:
        key = (F32, v)
        if key not in nc.const_aps.aps:
            t = nc.alloc_sbuf_tensor(f"constb-{v}", [128, 1], F32)
            nc.vector.memset(t.ap(), v)
            nc.const_aps.aps[key] = t.ap()
    # Only ACT reads the const APs (activation bias pointers); a narrow
    # DVE->Activation barrier keeps both DMA queues (sync, gpsimd) free to
    # start streaming immediately.
    nc.multi_engine_barrier([mybir.EngineType.DVE, mybir.EngineType.Activation])

    xt = nc.dram_tensor("xt", [I, BS], BF16, kind="ExternalInput")
    wsp = nc.dram_tensor("wsp", [NPAIRS * 128, 2 * O], FP8, kind="ExternalInput")
    wres = nc.dram_tensor("wres", [I, O], BF16, kind="ExternalInput")
    out = nc.dram_tensor("out", [BS, O], BF16, kind="ExternalOutput")

    with tile.TileContext(nc) as tc, ExitStack() as ctx:
        xpool = ctx.enter_context(tc.tile_pool(name="x", bufs=1))
        fpool = ctx.enter_context(tc.tile_pool(name="f", bufs=5))
        rpool = ctx.enter_context(tc.tile_pool(name="r", bufs=1))
        tpool = ctx.enter_context(tc.tile_pool(name="t", bufs=2))
        dpool = ctx.enter_context(tc.tile_pool(name="d", bufs=2))
        gpool = ctx.enter_context(tc.tile_pool(name="g", bufs=2))
        wspool = ctx.enter_context(tc.tile_pool(name="ws", bufs=10))
        wrpool = ctx.enter_context(tc.tile_pool(name="wr", bufs=8))
        opool = ctx.enter_context(tc.tile_pool(name="o", bufs=8))
        pspool = ctx.enter_context(tc.tile_pool(name="ps", bufs=1, space="PSUM"))

        # ---- x load: per-chunk f32 staging; bf16 clamped copy (xcb) on DVE;
        # bf16 residual plane cast from the staging tile via SWDGE (x is read
        # from HBM exactly once) ----
        # PE clock (HAM) warmup: ~4us of junk matmuls on memset scratch so
        # the gate is at full rate when the real stream starts. ps[0] is
        # reset by the first real matmul (start=True).
        sj = xpool.tile([128, 128], BF16, tag="sj")
        wj = xpool.tile([128, 512], BF16, tag="wj")
        nc.vector.memset(sj[:], 0.0)
        nc.vector.memset(wj[:], 0.0)

        # x ships as bf16 (everything downstream is bf16): ONE 1MB HBM read
        # into fres, which doubles as the residual plane; xcb clamps read it
        # in-SBUF. Startup HBM is ~1.5MB instead of 4.5MB (the old fp32
        # staging + re-read competed for the DMA wire and pushed the last
        # x-chunk clamp to t=22us).
        xcb = xpool.tile([128, FD], BF16, tag="xcb")
        fres = rpool.tile([128, NCH, BS], BF16, tag="fres")
        wrts = {}
        for c in range(NCH):
            sl = slice(c * BS, (c + 1) * BS)
            nc.sync.dma_start(fres[:, c:c + 1, :], xt[c * 128:(c + 1) * 128, :])
            if c < 2:
                wt = wrpool.tile([128, O], BF16, tag="wr", name=f"wr{c}")
                nc.gpsimd.dma_start(wt[:], wres[c * 128:(c + 1) * 128, :])
                wrts[c] = wt
            nc.vector.tensor_scalar(xcb[:, sl], fres[:, c:c + 1, :],
                                    -1.0, 1.0, AL.max, AL.min)
        for c in range(2, NCH):
            wt = wrpool.tile([128, O], BF16, tag="wr", name=f"wr{c}")
            nc.sync.dma_start(wt[:], wres[c * 128:(c + 1) * 128, :])
            wrts[c] = wt

        # ---- 11 spline planes -> fp8: u = 4*xcb+(5-k); q = u*u;
        # p = Exp(-alpha*q + lnA); SWDGE cast to fp8 ----
        PERM = list(range(K))
        planes = {}
        for k in PERM:
            fk = fpool.tile([128, NCH, BS], FP8, tag="fk", name=f"fk{k}")
            A, alpha = GAUSS[k]
            beta = math.log(A)
            cs = float(5 - k)
            nsub = NSUB.get(k, 1)
            sw = FD // nsub
            u = tpool.tile([128, FD], BF16, tag="u", name="u")
            q = dpool.tile([128, FD], BF16, tag="q", name="q")
            p = gpool.tile([128, FD], BF16, tag="p", name="p")
            for su in range(nsub):
                sl = slice(su * sw, (su + 1) * sw)
                csl = slice(su * (NCH // nsub), (su + 1) * (NCH // nsub))
                nc.vector.tensor_scalar(u[:, sl], xcb[:, sl], 4.0, cs,
                                        AL.mult, AL.add)
                nc.vector.tensor_tensor(q[:, sl], u[:, sl], u[:, sl], AL.mult)
                nc.scalar.activation(p[:, sl], q[:, sl], AF.Exp,
                                     bias=beta, scale=-alpha)
                nc.gpsimd.dma_start(fk[:, csl, :], p[:, sl])
            planes[k] = fk

        # ---- matmul: [128 b x 512 o] x (4 bc x 2 oh) = 8 PSUM banks.
        # 8 residual chunks cover PE warmup while the first planes build. ----
        ps = [pspool.tile([128, 512], F32, name=f"ps{j}", tag=f"ps{j}")
              for j in range(8)]

        for wi in range(18):
            nc.tensor.matmul(ps[0][:], sj[:], wj[:], start=(wi == 0),
                             stop=(wi == 17))

        def resid_mms(c, start):
            wt = wrts[c]
            for bc in range(4):
                lhsT = fres[:, c:c + 1, bc * 128:(bc + 1) * 128]
                for oh in range(2):
                    nc.tensor.matmul(ps[bc * 2 + oh][:], lhsT,
                                     wt[:, oh * 512:(oh + 1) * 512],
                                     start=start, stop=False)

        for c in range(NCH):
            resid_mms(c, c == 0)
        for pos, kk in enumerate(PERM):
            for cp in range(NCH // 2):
                j = kk * (NCH // 2) + cp
                last = (pos == len(PERM) - 1) and (cp == NCH // 2 - 1)
                wt = wspool.tile([128, 2, O], FP8, tag="ws")
                nc.sync.dma_start(
                    wt[:], wsp[j * 128:(j + 1) * 128, :]
                    .rearrange("p (two o) -> p two o", two=2))
                src = planes[kk]
                for bc in range(4):
                    lhsT = src[:, 2 * cp:2 * cp + 2, bc * 128:(bc + 1) * 128]
                    for oh in range(2):
                        nc.tensor.matmul(ps[bc * 2 + oh][:], lhsT,
                                         wt[:, :, oh * 512:(oh + 1) * 512],
                                         start=False, stop=last,
                                         perf_mode=DR)

        # ---- epilogue: PSUM * (1/SCALE) -> SBUF -> HBM. Ops split ACT/DVE,
        # out DMAs split across the sync and scalar HWDGE queues. ----
        for bc in range(4):
            for oh in range(2):
                obh = opool.tile([128, 512], BF16, tag="ob", name=f"ob{bc}{oh}")
                if oh == 0:
                    nc.scalar.mul(obh[:], ps[bc * 2 + oh][:], 1.0 / SCALE)
                else:
                    nc.vector.tensor_scalar(obh[:], ps[bc * 2 + oh][:],
                                            1.0 / SCALE, None, AL.mult)
                eng = nc.sync if oh == 0 else nc.scalar
                eng.dma_start(
                    out[bc * 128:(bc + 1) * 128, oh * 512:(oh + 1) * 512],
                    obh[:])

    nc.compile()
    _dedupe_ldweights(nc, mybir)
    return nc


def _dedupe_ldweights(nc, mybir):
    """Drop an Ldweights that reloads the exact same weights as the previous
    Ldweights on the PE stream with only Matmults in between (the oh=0/oh=1
    pair shares its stationary operand). Bail on any with sync_info."""
    import json as _json
    for fn in nc.m.functions:
        for blk in fn.blocks:
            insts = list(blk.instructions)
            kept = []
            last_key = None
            removed = 0
            for inst in insts:
                if inst.engine != mybir.EngineType.PE:
                    kept.append(inst)
                    continue
                op = type(inst).__name__
                if op == "InstLdweights":
                    si = inst.sync_info
                    has_sync = bool(si and (si.on_wait or si.on_update))
                    key = _json.dumps(
                        _json.loads(mybir.instruction_to_pretty_json_string(inst))
                        .get("ins"), sort_keys=True)
                    if key == last_key and not has_sync:
                        removed += 1
                        continue
                    last_key = key
                    kept.append(inst)
                elif op == "InstMatmult":
                    kept.append(inst)
                else:
                    last_key = None
                    kept.append(inst)
            if removed:
                blk.instructions = kept
    return nc


def _fold_weights(spline_coeffs, residual_weight, residual_bias, scale_base):
    scale = scale_base.astype(np.float32).mean(axis=1)                # [O]
    Ws = spline_coeffs.astype(np.float32) * scale[:, None, None]      # [O,I,K]
    Ws += residual_bias.astype(np.float32)[:, None, None] / I
    Ws *= SCALE / 6.0             # device planes are ~6*basis_k
    Ws = np.ascontiguousarray(Ws.transpose(2, 1, 0))                  # [K,I,O]
    # pair layout for DoubleRow moving operand: [44, 128, 2, O] rows
    Wsp = Ws.reshape(NPAIRS, 2, 128, O).transpose(0, 2, 1, 3).reshape(NPAIRS * 128, 2 * O)
    Wsp = np.clip(Wsp, -240.0, 240.0).astype(F8)
    Wr = np.ascontiguousarray(residual_weight.astype(np.float32).T * SCALE)
    Wr = Wr.astype(ml_dtypes.bfloat16)                                # [I,O]
    return Wsp, Wr


def _make_in_maps(inputs):
    Wsp, Wr = _fold_weights(inputs["spline_coeffs"], inputs["residual_weight"],
                            inputs["residual_bias"], inputs["scale_base"])
    x = np.asarray(inputs["x"], dtype=np.float32).astype(ml_dtypes.bfloat16)
    in_maps = []
    for c in range(NCORES):
        xs = np.ascontiguousarray(x[c * BS:(c + 1) * BS, :].T)  # [I, BS]
        in_maps.append({"xt": xs, "wsp": Wsp, "wres": Wr})
    return in_maps


def kernel(x, spline_coeffs, residual_weight, residual_bias, scale_base):
    from concourse.bass_utils import run_bass_kernel_spmd

    if "nc" not in _cache:
        _cache["nc"] = _build_bass()
    nc = _cache["nc"]

    in_maps = _make_in_maps(dict(x=x, spline_coeffs=spline_coeffs,
                                 residual_weight=residual_weight,
                                 residual_bias=residual_bias,
                                 scale_base=scale_base))
    res = run_bass_kernel_spmd(nc, in_maps, core_ids=list(range(NCORES)))
    out = np.concatenate([r["out"] for r in res.results], axis=0)
    return out.astype(np.float32)
